# revision 33
# baseline (speedup 1.0000x reference)
"""Trainium2 Bass kernel for nn_NeRF_MLP_Compose (MoE-routed NeRF MLP).

Strategy (v2):
  - Host-side MoE dispatch: each expert's rows are split across a PAIR of
    cores (core c handles expert c//2), so each core runs ONE expert dense
    over ~8.2k rows (CAP=8704 padded) and holds only that expert's weights.
  - bf16 weights + activations for all matmuls (tolerance is 2e-2); the
    positional-encoding angle path stays fp32 for phase accuracy.
  - Row-major front-end: normalize + angle/[mod 1]/sin/cos are computed with
    rows on partitions (no PE transposes, no PSUM copies); the encoded
    features are flipped to feature-major with the DMA XBAR transpose
    (16-bit, 16x128 tiles).  Output is flipped back the same way.
  - MLP: feature-major, K<=128 stationary blocks, N=512 moving tiles.
    Third residual folded into the out layer input (h3 = s2*t3 + h2).
  - Element-wise work is spread across ACT / DVE / GPSIMD so each engine's
    per-tile time roughly matches the PE's; the Tile list-scheduler
    overlaps tiles (all pools are multi-buffered).
"""
import sys
for _p in ("/opt/trn_rl_repo", "/root/.axon_site/_ro/trn_rl_repo"):
    if _p not in sys.path:
        sys.path.insert(0, _p)

import numpy as np
import ml_dtypes

N = 65536
E = 4            # experts
NCORE = 8
CAP = 8704       # rows per core (one expert per core pair; 2*CAP=17408 >> E[16384])
NT = 17          # 512-row tiles per core
R = 512          # rows per tile
C = 4            # 128-row chunks per tile
NUM_FREQS = 10
HID = 256
DOUT = 64
NL = 4           # layers -> 3 residual blocks
TWO_PI_F32 = float(np.float32(2 * np.pi))
HALF_PI_F32 = float(np.float32(0.5 * np.pi))
MAGIC_C = float(np.float32(1.5 * 2 ** 23))

_compiled = {}
RUN_KWARGS = {}    # test.py may set e.g. {"trace": True}
LAST_RESULT = []   # test.py reads the BassKernelResults appended here


def _build_program():
    import concourse.bass as bass
    from concourse import bacc
    import concourse.mybir as mybir
    import concourse.tile as tile

    F32 = mybir.dt.float32
    BF16 = mybir.dt.bfloat16
    P = 128
    ALU = mybir.AluOpType
    ACTF = mybir.ActivationFunctionType

    nc = bacc.Bacc("TRN2", target_bir_lowering=False, debug=False)

    # ---- DRAM I/O (per core; one expert's weights) ----
    x_d = nc.dram_tensor("x_rows", [CAP, 4], F32, kind="ExternalInput").ap()
    d_d = nc.dram_tensor("indim_rows", [CAP], F32, kind="ExternalInput").ap()
    fr_d = nc.dram_tensor("fr10", [NUM_FREQS], F32, kind="ExternalInput").ap()
    w0_d = nc.dram_tensor("w0", [85, HID], BF16, kind="ExternalInput").ap()
    wh_d = nc.dram_tensor("wh", [P, NL - 1, 2, 2, P], BF16,
                          kind="ExternalInput").ap()
    wo_d = nc.dram_tensor("wo", [P, 2, DOUT], BF16, kind="ExternalInput").ap()
    b0_d = nc.dram_tensor("b0r", [P, 2], F32, kind="ExternalInput").ap()
    bh_d = nc.dram_tensor("bhr", [P, NL - 1, 2], F32, kind="ExternalInput").ap()
    bo_d = nc.dram_tensor("bor", [DOUT, 1], F32, kind="ExternalInput").ap()
    sc_d = nc.dram_tensor("scal3", [NL - 1], F32, kind="ExternalInput").ap()
    out_d = nc.dram_tensor("out_rows", [CAP, DOUT], F32,
                           kind="ExternalOutput").ap()

    with tile.TileContext(nc) as tc:
        with tc.tile_pool(name="const", bufs=1) as cpool, \
             tc.tile_pool(name="fr", bufs=4) as fpool, \
             tc.tile_pool(name="act", bufs=8) as apool, \
             tc.tile_pool(name="psz", bufs=3, space="PSUM") as zpool, \
             tc.tile_pool(name="pso", bufs=2, space="PSUM") as opool:

            # ---- front-end constants first (the first tiles' front
            # chain must not queue behind the big weight DMAs) ----
            scl = cpool.tile([P, NL - 1], F32)
            nc.scalar.dma_start(
                out=scl,
                in_=bass.AP(tensor=sc_d.tensor, offset=0,
                            ap=[[0, P], [1, NL - 1]]))
            fr = cpool.tile([P, NUM_FREQS], F32)
            nc.scalar.dma_start(
                out=fr,
                in_=bass.AP(tensor=fr_d.tensor, offset=0,
                            ap=[[0, P], [1, NUM_FREQS]]))
            ph = cpool.tile([P, 2], F32)
            nc.vector.memset(ph[:, 0:1], 0.0)
            nc.vector.memset(ph[:, 1:2], 0.25)
            w0 = cpool.tile([85, HID], BF16)
            wh = cpool.tile([P, NL - 1, 2, 2, P], BF16)
            wo = cpool.tile([P, 2, DOUT], BF16)
            wos = cpool.tile([P, 2, DOUT], BF16)
            b0 = cpool.tile([P, 2], F32)
            bh = cpool.tile([P, NL - 1, 2], F32)
            bo = cpool.tile([DOUT, 1], F32)

            def load_weights():
                nc.scalar.dma_start(out=w0, in_=w0_d)
                nc.scalar.dma_start(out=wh, in_=wh_d)
                nc.scalar.dma_start(out=wo, in_=wo_d)
                nc.scalar.dma_start(out=b0, in_=b0_d)
                nc.scalar.dma_start(out=bh, in_=bh_d)
                nc.scalar.dma_start(out=bo, in_=bo_d)
                # s2-prescaled out weights: out = Wo^T h2 + (s2 Wo)^T t3 --
                # removes the third residual STT from the per-tile loop
                nc.vector.tensor_scalar_mul(wos, wo, scl[:, 2:3])

            def fronts(group):
                """Group-merged row-major front-end: one op per stage for
                the whole group (fewer instructions, less fixed overhead)."""
                G = len(group)
                CG = G * C
                r0 = group[0] * R
                x_g = fpool.tile([P, CG, 4], F32, tag="x_t", bufs=2)
                nc.sync.dma_start(
                    out=x_g,
                    in_=bass.AP(tensor=x_d.tensor, offset=r0 * 4,
                                ap=[[4, P], [4 * P, CG], [1, 4]]))
                d_g = fpool.tile([P, CG], F32, tag="d_t", bufs=2)
                nc.sync.dma_start(
                    out=d_g,
                    in_=bass.AP(tensor=d_d.tensor, offset=r0,
                                ap=[[1, P], [P, CG]]))

                rc = fpool.tile([P, CG], F32, tag="rc")
                nc.vector.reciprocal(rc, x_g[:, :, 3])
                xn = fpool.tile([P, CG, 4], F32, tag="xn")
                nc.gpsimd.tensor_mul(xn, x_g,
                                     rc[:, :, None].to_broadcast((P, CG, 4)))
                nc.gpsimd.tensor_copy(xn[:, :, 3], x_g[:, :, 3])

                # angles in turns: t20[p, c, j, i] = x'_j * 2^(i-1) (exact);
                # t40 doubles it with the cos quarter-turn phase (folded in
                # BEFORE range reduction -- the Sin table domain is ~[-pi,pi])
                t20 = fpool.tile([P, CG, 4, NUM_FREQS], F32, tag="t20")
                nc.gpsimd.tensor_mul(
                    t20, xn[:, :, :, None].to_broadcast((P, CG, 4, NUM_FREQS)),
                    fr[:, None, None, :].to_broadcast((P, CG, 4, NUM_FREQS)))
                t20f = t20.rearrange("p c j i -> p c (j i)")
                t40 = fpool.tile([P, CG, 2, 40], F32, tag="t40")
                nc.gpsimd.tensor_tensor(
                    t40,
                    t20f[:, :, None, :].to_broadcast((P, CG, 2, 40)),
                    ph[:, None, :, None].to_broadcast((P, CG, 2, 40)),
                    ALU.add)
                # k = round(t40) via fp32 magic add; m40 = t40 - k in [-.5,.5]
                kt = fpool.tile([P, CG, 2, 40], F32, tag="kt")
                nc.vector.tensor_scalar(kt, t40, MAGIC_C, MAGIC_C,
                                        ALU.add, ALU.subtract)
                m40 = fpool.tile([P, CG, 2, 40], F32, tag="m40")
                nc.gpsimd.tensor_tensor(m40, t40, kt, ALU.subtract)

                # xe rows: [0:4]=x', [4:44]=sin, [44:84]=cos, [84]=1 (bias
                # row for the l0 matmul), [85:128]=junk
                xe_r = fpool.tile([P, CG, P], BF16, tag="xe_r")
                nc.gpsimd.tensor_copy(xe_r[:, :, 0:4], xn)
                nc.gpsimd.memset(xe_r[:, :, 84:85], 1.0)
                m40f = m40.rearrange("p c s f -> p c (s f)")
                nc.scalar.activation(xe_r[:, :, 4:84], m40f, ACTF.Sin,
                                     bias=0.0, scale=TWO_PI_F32)

                # flip to feature-major via one DMA XBAR transpose
                xe_g = apool.tile([P, G * R], BF16, tag="xe")
                nc.sync.dma_start(
                    out=xe_g.rearrange("p (c q) -> p c q", c=CG),
                    in_=xe_r, transpose=True)
                for gi, t in enumerate(group):
                    st[t] = {"xe": xe_g[:, gi * R:(gi + 1) * R],
                             "dg": d_g, "gi": gi}

            def l0_mm(xe):
                # bias rides the ones row, so one biasless relu covers both
                # halves of z0
                z0 = zpool.tile([P, 2, R], F32, tag="z")
                nc.tensor.matmul(z0[:, 0, :], w0[:, 0:P], xe[0:85, :],
                                 start=True, stop=True)
                nc.tensor.matmul(z0[:, 1, :], w0[:, P:HID], xe[0:85, :],
                                 start=True, stop=True)
                return z0

            def l0_relu(z0):
                h = apool.tile([P, 2, R], BF16, tag="h")
                nc.scalar.activation(h, z0, ACTF.Relu, bias=0.0, scale=1.0)
                return h

            def layer_mm(k, h):
                zk = zpool.tile([P, 2, R], F32, tag="z")
                for mb in range(2):
                    nc.tensor.matmul(zk[:, mb, :], wh[:, k, 0, mb, :],
                                     h[:, 0, :], start=True, stop=False)
                    nc.tensor.matmul(zk[:, mb, :], wh[:, k, 1, mb, :],
                                     h[:, 1, :], start=False, stop=True)
                return zk

            def layer_post(k, zk, h):
                # t = relu(zk + bh);  k<2: h' = s_k t + h;  k==2: keep t3
                # (its residual is folded into the prescaled out weights)
                tt = apool.tile([P, 2, R], BF16, tag="t")
                nc.scalar.activation(tt[:, 0, :], zk[:, 0, :], ACTF.Relu,
                                     bias=bh[:, k, 0:1], scale=1.0)
                if k == 1:
                    nc.scalar.activation(tt[:, 1, :], zk[:, 1, :], ACTF.Relu,
                                         bias=bh[:, k, 1:2], scale=1.0)
                else:
                    nc.vector.tensor_scalar(tt[:, 1, :], zk[:, 1, :],
                                            bh[:, k, 1:2], 0.0,
                                            ALU.add, ALU.max)
                if k == 2:
                    return h, tt
                h_new = apool.tile([P, 2, R], BF16, tag="h")
                nc.vector.scalar_tensor_tensor(h_new, tt, scl[:, k:k + 1],
                                               h, ALU.mult, ALU.add)
                return h_new, None

            def out_mm_h2(h2):
                o_ps = opool.tile([DOUT, R], F32, tag="o")
                nc.tensor.matmul(o_ps, wo[:, 0, :], h2[:, 0, :],
                                 start=True, stop=False)
                nc.tensor.matmul(o_ps, wo[:, 1, :], h2[:, 1, :],
                                 start=False, stop=False)
                return o_ps

            def out_mm_t3(o_ps, t3):
                nc.tensor.matmul(o_ps, wos[:, 0, :], t3[:, 0, :],
                                 start=False, stop=False)
                nc.tensor.matmul(o_ps, wos[:, 1, :], t3[:, 1, :],
                                 start=False, stop=True)

            def epilogue(t, o_ps, d2):
                r0 = t * R
                oT = fpool.tile([DOUT, R], BF16, tag="oT")
                nc.scalar.activation(oT, o_ps, ACTF.Identity,
                                     bias=bo, scale=1.0)
                # flip back to row-major, divide by in_dim, store
                o_r = fpool.tile([P, C, DOUT], BF16, tag="o_r")
                nc.sync.dma_start(out=o_r, in_=oT, transpose=True)
                rid = fpool.tile([P, C], F32, tag="rid")
                nc.vector.reciprocal(rid, d2)
                o_f = fpool.tile([P, C, DOUT], F32, tag="o_f")
                nc.gpsimd.tensor_mul(
                    o_f, o_r, rid[:, :, None].to_broadcast((P, C, DOUT)))
                nc.sync.dma_start(
                    out=bass.AP(tensor=out_d.tensor, offset=r0 * DOUT,
                                ap=[[DOUT, P], [P * DOUT, C], [1, DOUT]]),
                    in_=o_f)

            # two/three tiles interleaved per layer so the PE always has a
            # ready matmul burst while another tile's relu/residual chain
            # runs; the next group's front-end is emitted before this
            # group's hidden layers to fill engine idle
            groups = [[t0, t0 + 1] for t0 in range(0, NT - 3, 2)]
            groups.append(list(range(NT - 3, NT)))  # last: 3-way interleave

            st = {}
            fronts(groups[0])
            load_weights()
            for pi, group in enumerate(groups):
                for t in group:
                    st[t]["z"] = l0_mm(st[t]["xe"])
                for t in group:
                    st[t]["h"] = l0_relu(st[t]["z"])
                if pi + 1 < len(groups):
                    fronts(groups[pi + 1])
                for k in range(NL - 1):
                    for t in group:
                        st[t]["zk"] = layer_mm(k, st[t]["h"])
                    if k == 2:
                        # out-layer h2 part fills the PE gap while the k2
                        # relu chain runs on ACT/DVE
                        for t in group:
                            st[t]["o"] = out_mm_h2(st[t]["h"])
                    for t in group:
                        st[t]["h"], st[t]["t3"] = layer_post(
                            k, st[t]["zk"], st[t]["h"])
                for t in group:
                    out_mm_t3(st[t]["o"], st[t]["t3"])
                for t in group:
                    gi = st[t]["gi"]
                    d2 = st[t]["dg"][:, gi * C:(gi + 1) * C]
                    epilogue(t, st[t]["o"], d2)

    nc.compile()
    return nc


def _get_program():
    if "nc" not in _compiled:
        _compiled["nc"] = _build_program()
    return _compiled["nc"]


def _xe_perm():
    """perm[slot] = reference xe column for device slot order
    (slots: 0..3 = x', 4 + j*10 + i = sin, 44 + j*10 + i = cos)."""
    perm = np.zeros(84, np.int64)
    perm[0:4] = np.arange(4)
    for s in range(2):
        for j in range(4):
            for i in range(NUM_FREQS):
                perm[4 + s * 40 + j * 10 + i] = 4 + i * 8 + j * 2 + s
    return perm


def _prep_weights(e, W0, b0, Wh, bh, scal, Wout, bout):
    """Host-side layout transforms (permutation / reshape / cast only)."""
    bf = ml_dtypes.bfloat16
    w0 = np.ascontiguousarray(
        np.vstack([W0[e][_xe_perm()], b0[e][None, :]])).astype(bf)  # [85,256]
    wh = np.ascontiguousarray(
        Wh[e].reshape(NL - 1, 2, 128, 2, 128)
        .transpose(2, 0, 1, 3, 4)).astype(bf)                      # [128,3,2,2,128]
    wo = np.ascontiguousarray(
        Wout[e].reshape(2, 128, DOUT).transpose(1, 0, 2)).astype(bf)
    b0r = np.ascontiguousarray(b0[e].reshape(2, 128).T)            # [128,2]
    bhr = np.ascontiguousarray(
        bh[e].reshape(NL - 1, 2, 128).transpose(2, 0, 1))          # [128,3,2]
    bor = np.ascontiguousarray(bout[e].reshape(DOUT, 1))
    sc3 = np.ascontiguousarray(scal[e])
    fr10 = (2.0 ** (np.arange(NUM_FREQS, dtype=np.float32) - 1.0)).astype(
        np.float32)
    return dict(w0=w0, wh=wh, wo=wo, b0r=b0r, bhr=bhr, bor=bor,
                scal3=sc3, fr10=fr10)


def kernel(x, in_dim, layer_id, W0, b0, Wh, bh, scal, Wout, bout):
    from concourse.bass_utils import run_bass_kernel_spmd

    x = np.asarray(x, np.float32)
    in_dim = np.asarray(in_dim, np.float32)
    layer_id = np.asarray(layer_id)
    W0 = np.asarray(W0, np.float32)
    b0 = np.asarray(b0, np.float32)
    Wh = np.asarray(Wh, np.float32)
    bh = np.asarray(bh, np.float32)
    scal = np.asarray(scal, np.float32)
    Wout = np.asarray(Wout, np.float32)
    bout = np.asarray(bout, np.float32)

    # ---- dispatch: expert e -> cores 2e, 2e+1; pad to CAP per core ----
    PADIDX = N
    x_aug = np.vstack([x, np.ones((1, 4), np.float32)])
    d_aug = np.concatenate([in_dim, np.ones(1, np.float32)])
    perms = np.full((NCORE, CAP), PADIDX, np.int64)
    overflow = []
    for e in range(E):
        idx = np.flatnonzero(layer_id == e)
        if len(idx) > 2 * CAP:
            overflow.append(idx[2 * CAP:])
            idx = idx[:2 * CAP]
        nh = min((len(idx) + 1) // 2, CAP)
        perms[2 * e, :nh] = idx[:nh]
        perms[2 * e + 1, :len(idx) - nh] = idx[nh:]

    in_maps = []
    for c in range(NCORE):
        m = _prep_weights(c // 2, W0, b0, Wh, bh, scal, Wout, bout)
        p = perms[c]
        m["x_rows"] = np.ascontiguousarray(x_aug[p])
        m["indim_rows"] = np.ascontiguousarray(d_aug[p])
        in_maps.append(m)

    nc = _get_program()
    res = run_bass_kernel_spmd(nc, in_maps, core_ids=list(range(NCORE)),
                               **RUN_KWARGS)
    LAST_RESULT.clear()
    LAST_RESULT.append(res)

    out = np.zeros((N + 1, DOUT), np.float32)
    for c in range(NCORE):
        out[perms[c]] = np.asarray(res.results[c]["out_rows"], np.float32)

    # pathological overflow fallback (never hit for the benchmark input)
    if overflow:
        ov = np.concatenate(overflow)
        out[ov] = _numpy_ref(x[ov], in_dim[ov], layer_id[ov], W0, b0, Wh, bh,
                             scal, Wout, bout)
    return out[:N]


def _numpy_ref(x, in_dim, layer_id, W0, b0, Wh, bh, scal, Wout, bout):
    x = np.concatenate([x[:, :3] / x[:, 3:4], x[:, 3:]], axis=1)
    freqs = (2.0 ** np.arange(NUM_FREQS, dtype=np.float32)) * np.float32(np.pi)
    ang = x[:, None, :] * freqs[None, :, None]
    sc = np.stack([np.sin(ang), np.cos(ang)], axis=-1)
    xe = np.concatenate([x, sc.reshape(x.shape[0], -1)], axis=1)
    out = np.zeros((x.shape[0], DOUT), np.float32)
    for e in range(E):
        m = layer_id == e
        if not m.any():
            continue
        h = np.maximum(xe[m] @ W0[e] + b0[e], 0.0)
        for k in range(NL - 1):
            h = scal[e, k] * np.maximum(h @ Wh[e, k] + bh[e, k], 0.0) + h
        out[m] = h @ Wout[e] + bout[e]
    return out / in_dim[:, None]


# revision 34
# speedup vs baseline: 1.0805x; 1.0805x over previous
"""Trainium2 Bass kernel for nn_NeRF_MLP_Compose (MoE-routed NeRF MLP).

Strategy (v2):
  - Host-side MoE dispatch: each expert's rows are split across a PAIR of
    cores (core c handles expert c//2), so each core runs ONE expert dense
    over ~8.2k rows (CAP=8704 padded) and holds only that expert's weights.
  - bf16 weights + activations for all matmuls (tolerance is 2e-2); the
    positional-encoding angle path stays fp32 for phase accuracy.
  - Row-major front-end: normalize + angle/[mod 1]/sin/cos are computed with
    rows on partitions (no PE transposes, no PSUM copies); the encoded
    features are flipped to feature-major with the DMA XBAR transpose
    (16-bit, 16x128 tiles).  Output is flipped back the same way.
  - MLP: feature-major, K<=128 stationary blocks, N=512 moving tiles.
    Third residual folded into the out layer input (h3 = s2*t3 + h2).
  - Element-wise work is spread across ACT / DVE / GPSIMD so each engine's
    per-tile time roughly matches the PE's; the Tile list-scheduler
    overlaps tiles (all pools are multi-buffered).
"""
import sys
for _p in ("/opt/trn_rl_repo", "/root/.axon_site/_ro/trn_rl_repo"):
    if _p not in sys.path:
        sys.path.insert(0, _p)

import numpy as np
import ml_dtypes

N = 65536
E = 4            # experts
NCORE = 8
CAP = 8704       # rows per core (one expert per core pair; 2*CAP=17408 >> E[16384])
NT = 17          # 512-row tiles per core
R = 512          # rows per tile
C = 4            # 128-row chunks per tile
NUM_FREQS = 10
HID = 256
DOUT = 64
NL = 4           # layers -> 3 residual blocks
TWO_PI_F32 = float(np.float32(2 * np.pi))
HALF_PI_F32 = float(np.float32(0.5 * np.pi))
MAGIC_C = float(np.float32(1.5 * 2 ** 23))

_compiled = {}
RUN_KWARGS = {}    # test.py may set e.g. {"trace": True}
LAST_RESULT = []   # test.py reads the BassKernelResults appended here


def _build_program():
    import concourse.bass as bass
    from concourse import bacc
    import concourse.mybir as mybir
    import concourse.tile as tile

    F32 = mybir.dt.float32
    BF16 = mybir.dt.bfloat16
    P = 128
    ALU = mybir.AluOpType
    ACTF = mybir.ActivationFunctionType

    nc = bacc.Bacc("TRN2", target_bir_lowering=False, debug=False)

    # ---- DRAM I/O (per core; one expert's weights) ----
    x_d = nc.dram_tensor("x_rows", [CAP, 4], F32, kind="ExternalInput").ap()
    d_d = nc.dram_tensor("indim_rows", [CAP], F32, kind="ExternalInput").ap()
    fr_d = nc.dram_tensor("fr10", [NUM_FREQS], F32, kind="ExternalInput").ap()
    w0_d = nc.dram_tensor("w0", [85, HID], BF16, kind="ExternalInput").ap()
    wh_d = nc.dram_tensor("wh", [P, NL - 1, 2, 2, P], BF16,
                          kind="ExternalInput").ap()
    wo_d = nc.dram_tensor("wo", [P, 2, DOUT], BF16, kind="ExternalInput").ap()
    b0_d = nc.dram_tensor("b0r", [P, 2], F32, kind="ExternalInput").ap()
    bh_d = nc.dram_tensor("bhr", [P, NL - 1, 2], F32, kind="ExternalInput").ap()
    bo_d = nc.dram_tensor("bor", [DOUT, 1], F32, kind="ExternalInput").ap()
    sc_d = nc.dram_tensor("scal3", [NL - 1], F32, kind="ExternalInput").ap()
    out_d = nc.dram_tensor("out_rows", [CAP, DOUT], F32,
                           kind="ExternalOutput").ap()

    with tile.TileContext(nc) as tc:
        with tc.tile_pool(name="const", bufs=1) as cpool, \
             tc.tile_pool(name="fr", bufs=4) as fpool, \
             tc.tile_pool(name="act", bufs=8) as apool, \
             tc.tile_pool(name="psz", bufs=3, space="PSUM") as zpool, \
             tc.tile_pool(name="pso", bufs=2, space="PSUM") as opool:

            # ---- front-end constants first (the first tiles' front
            # chain must not queue behind the big weight DMAs) ----
            scl = cpool.tile([P, NL - 1], F32)
            nc.scalar.dma_start(
                out=scl,
                in_=bass.AP(tensor=sc_d.tensor, offset=0,
                            ap=[[0, P], [1, NL - 1]]))
            fr = cpool.tile([P, NUM_FREQS], F32)
            nc.scalar.dma_start(
                out=fr,
                in_=bass.AP(tensor=fr_d.tensor, offset=0,
                            ap=[[0, P], [1, NUM_FREQS]]))
            ph = cpool.tile([P, 2], F32)
            nc.vector.memset(ph[:, 0:1], 0.0)
            nc.vector.memset(ph[:, 1:2], 0.25)
            w0 = cpool.tile([85, HID], BF16)
            wh = cpool.tile([P, NL - 1, 2, 2, P], BF16)
            wo = cpool.tile([P, 2, DOUT], BF16)
            wos = cpool.tile([P, 2, DOUT], BF16)
            b0 = cpool.tile([P, 2], F32)
            bh = cpool.tile([P, NL - 1, 2], F32)
            bo = cpool.tile([DOUT, 1], F32)

            def load_weights():
                nc.scalar.dma_start(out=w0, in_=w0_d)
                nc.scalar.dma_start(out=wh, in_=wh_d)
                nc.scalar.dma_start(out=wo, in_=wo_d)
                nc.scalar.dma_start(out=b0, in_=b0_d)
                nc.scalar.dma_start(out=bh, in_=bh_d)
                nc.scalar.dma_start(out=bo, in_=bo_d)
                # s2-prescaled out weights: out = Wo^T h2 + (s2 Wo)^T t3 --
                # removes the third residual STT from the per-tile loop
                nc.vector.tensor_scalar_mul(wos, wo, scl[:, 2:3])

            def fronts(group):
                """Front-end for a group: one batched x/d load, then
                per-tile encode chains (small ops keep latency low)."""
                G = len(group)
                CG = G * C
                r0 = group[0] * R
                x_g = fpool.tile([P, CG, 4], F32, tag="x_t", bufs=2)
                nc.sync.dma_start(
                    out=x_g,
                    in_=bass.AP(tensor=x_d.tensor, offset=r0 * 4,
                                ap=[[4, P], [4 * P, CG], [1, 4]]))
                d_g = fpool.tile([P, CG], F32, tag="d_t", bufs=2)
                nc.sync.dma_start(
                    out=d_g,
                    in_=bass.AP(tensor=d_d.tensor, offset=r0,
                                ap=[[1, P], [P, CG]]))
                for gi, t in enumerate(group):
                    xe = front(x_g[:, gi * C:(gi + 1) * C, :])
                    st[t] = {"xe": xe, "dg": d_g, "gi": gi}

            def front(x_t):
                """Row-major front-end for one tile: normalize + encode."""
                rc = fpool.tile([P, C], F32, tag="rc")
                nc.vector.reciprocal(rc, x_t[:, :, 3])
                xn = fpool.tile([P, C, 4], F32, tag="xn")
                nc.gpsimd.tensor_mul(xn, x_t,
                                     rc[:, :, None].to_broadcast((P, C, 4)))
                nc.gpsimd.tensor_copy(xn[:, :, 3], x_t[:, :, 3])

                # angles in turns: t20[p, c, j, i] = x'_j * 2^(i-1) (exact);
                # t40 doubles it with the cos quarter-turn phase (folded in
                # BEFORE range reduction -- the Sin table domain is ~[-pi,pi])
                t20 = fpool.tile([P, C, 4, NUM_FREQS], F32, tag="t20")
                nc.gpsimd.tensor_mul(
                    t20, xn[:, :, :, None].to_broadcast((P, C, 4, NUM_FREQS)),
                    fr[:, None, None, :].to_broadcast((P, C, 4, NUM_FREQS)))
                t20f = t20.rearrange("p c j i -> p c (j i)")
                t40 = fpool.tile([P, C, 2, 40], F32, tag="t40")
                nc.gpsimd.tensor_tensor(
                    t40,
                    t20f[:, :, None, :].to_broadcast((P, C, 2, 40)),
                    ph[:, None, :, None].to_broadcast((P, C, 2, 40)),
                    ALU.add)
                # k = round(t40) via fp32 magic add; m40 = t40 - k in [-.5,.5]
                kt = fpool.tile([P, C, 2, 40], F32, tag="kt")
                nc.vector.tensor_scalar(kt, t40, MAGIC_C, MAGIC_C,
                                        ALU.add, ALU.subtract)
                m40 = fpool.tile([P, C, 2, 40], F32, tag="m40")
                nc.gpsimd.tensor_tensor(m40, t40, kt, ALU.subtract)

                # xe rows: [0:4]=x', [4:44]=sin, [44:84]=cos, [84]=1 (bias
                # row for the l0 matmul), [85:128]=junk
                xe_r = fpool.tile([P, C, P], BF16, tag="xe_r")
                nc.gpsimd.tensor_copy(xe_r[:, :, 0:4], xn)
                nc.gpsimd.memset(xe_r[:, :, 84:85], 1.0)
                m40f = m40.rearrange("p c s f -> p c (s f)")
                nc.scalar.activation(xe_r[:, :, 4:84], m40f, ACTF.Sin,
                                     bias=0.0, scale=TWO_PI_F32)

                # flip to feature-major via DMA XBAR transpose
                xe = apool.tile([P, R], BF16, tag="xe")
                nc.sync.dma_start(out=xe.rearrange("p (c q) -> p c q", c=C),
                                  in_=xe_r, transpose=True)
                return xe

            def l0_mm(xe):
                # bias rides the ones row, so one biasless relu covers both
                # halves of z0
                z0 = zpool.tile([P, 2, R], F32, tag="z")
                nc.tensor.matmul(z0[:, 0, :], w0[:, 0:P], xe[0:85, :],
                                 start=True, stop=True)
                nc.tensor.matmul(z0[:, 1, :], w0[:, P:HID], xe[0:85, :],
                                 start=True, stop=True)
                return z0

            def l0_relu(z0):
                h = apool.tile([P, 2, R], BF16, tag="h")
                nc.scalar.activation(h, z0, ACTF.Relu, bias=0.0, scale=1.0)
                return h

            def layer_mm(k, h):
                zk = zpool.tile([P, 2, R], F32, tag="z")
                for mb in range(2):
                    nc.tensor.matmul(zk[:, mb, :], wh[:, k, 0, mb, :],
                                     h[:, 0, :], start=True, stop=False)
                    nc.tensor.matmul(zk[:, mb, :], wh[:, k, 1, mb, :],
                                     h[:, 1, :], start=False, stop=True)
                return zk

            def layer_post(k, zk, h):
                # t = relu(zk + bh);  k<2: h' = s_k t + h;  k==2: keep t3
                # (its residual is folded into the prescaled out weights)
                tt = apool.tile([P, 2, R], BF16, tag="t")
                nc.scalar.activation(tt[:, 0, :], zk[:, 0, :], ACTF.Relu,
                                     bias=bh[:, k, 0:1], scale=1.0)
                if k == 1:
                    nc.scalar.activation(tt[:, 1, :], zk[:, 1, :], ACTF.Relu,
                                         bias=bh[:, k, 1:2], scale=1.0)
                else:
                    nc.vector.tensor_scalar(tt[:, 1, :], zk[:, 1, :],
                                            bh[:, k, 1:2], 0.0,
                                            ALU.add, ALU.max)
                if k == 2:
                    return h, tt
                h_new = apool.tile([P, 2, R], BF16, tag="h")
                nc.vector.scalar_tensor_tensor(h_new, tt, scl[:, k:k + 1],
                                               h, ALU.mult, ALU.add)
                return h_new, None

            def out_mm_h2(h2):
                o_ps = opool.tile([DOUT, R], F32, tag="o")
                nc.tensor.matmul(o_ps, wo[:, 0, :], h2[:, 0, :],
                                 start=True, stop=False)
                nc.tensor.matmul(o_ps, wo[:, 1, :], h2[:, 1, :],
                                 start=False, stop=False)
                return o_ps

            def out_mm_t3(o_ps, t3):
                nc.tensor.matmul(o_ps, wos[:, 0, :], t3[:, 0, :],
                                 start=False, stop=False)
                nc.tensor.matmul(o_ps, wos[:, 1, :], t3[:, 1, :],
                                 start=False, stop=True)

            def epilogue(t, o_ps, d2):
                r0 = t * R
                oT = fpool.tile([DOUT, R], BF16, tag="oT")
                nc.scalar.activation(oT, o_ps, ACTF.Identity,
                                     bias=bo, scale=1.0)
                # flip back to row-major, divide by in_dim, store
                o_r = fpool.tile([P, C, DOUT], BF16, tag="o_r")
                nc.scalar.dma_start(out=o_r, in_=oT, transpose=True)
                rid = fpool.tile([P, C], F32, tag="rid")
                nc.vector.reciprocal(rid, d2)
                o_f = fpool.tile([P, C, DOUT], F32, tag="o_f")
                nc.gpsimd.tensor_mul(
                    o_f, o_r, rid[:, :, None].to_broadcast((P, C, DOUT)))
                nc.scalar.dma_start(
                    out=bass.AP(tensor=out_d.tensor, offset=r0 * DOUT,
                                ap=[[DOUT, P], [P * DOUT, C], [1, DOUT]]),
                    in_=o_f)

            # two/three tiles interleaved per layer so the PE always has a
            # ready matmul burst while another tile's relu/residual chain
            # runs; the next group's front-end is emitted before this
            # group's hidden layers to fill engine idle
            groups = [[t0, t0 + 1] for t0 in range(0, NT - 3, 2)]
            groups.append(list(range(NT - 3, NT)))  # last: 3-way interleave

            st = {}
            fronts(groups[0])
            load_weights()
            for pi, group in enumerate(groups):
                for t in group:
                    st[t]["z"] = l0_mm(st[t]["xe"])
                for t in group:
                    st[t]["h"] = l0_relu(st[t]["z"])
                if pi + 1 < len(groups):
                    fronts(groups[pi + 1])
                for k in range(NL - 1):
                    for t in group:
                        st[t]["zk"] = layer_mm(k, st[t]["h"])
                    if k == 2:
                        # out-layer h2 part fills the PE gap while the k2
                        # relu chain runs on ACT/DVE
                        for t in group:
                            st[t]["o"] = out_mm_h2(st[t]["h"])
                    for t in group:
                        st[t]["h"], st[t]["t3"] = layer_post(
                            k, st[t]["zk"], st[t]["h"])
                for t in group:
                    out_mm_t3(st[t]["o"], st[t]["t3"])
                for t in group:
                    gi = st[t]["gi"]
                    d2 = st[t]["dg"][:, gi * C:(gi + 1) * C]
                    epilogue(t, st[t]["o"], d2)

    nc.compile()
    return nc


def _get_program():
    if "nc" not in _compiled:
        _compiled["nc"] = _build_program()
    return _compiled["nc"]


def _xe_perm():
    """perm[slot] = reference xe column for device slot order
    (slots: 0..3 = x', 4 + j*10 + i = sin, 44 + j*10 + i = cos)."""
    perm = np.zeros(84, np.int64)
    perm[0:4] = np.arange(4)
    for s in range(2):
        for j in range(4):
            for i in range(NUM_FREQS):
                perm[4 + s * 40 + j * 10 + i] = 4 + i * 8 + j * 2 + s
    return perm


def _prep_weights(e, W0, b0, Wh, bh, scal, Wout, bout):
    """Host-side layout transforms (permutation / reshape / cast only)."""
    bf = ml_dtypes.bfloat16
    w0 = np.ascontiguousarray(
        np.vstack([W0[e][_xe_perm()], b0[e][None, :]])).astype(bf)  # [85,256]
    wh = np.ascontiguousarray(
        Wh[e].reshape(NL - 1, 2, 128, 2, 128)
        .transpose(2, 0, 1, 3, 4)).astype(bf)                      # [128,3,2,2,128]
    wo = np.ascontiguousarray(
        Wout[e].reshape(2, 128, DOUT).transpose(1, 0, 2)).astype(bf)
    b0r = np.ascontiguousarray(b0[e].reshape(2, 128).T)            # [128,2]
    bhr = np.ascontiguousarray(
        bh[e].reshape(NL - 1, 2, 128).transpose(2, 0, 1))          # [128,3,2]
    bor = np.ascontiguousarray(bout[e].reshape(DOUT, 1))
    sc3 = np.ascontiguousarray(scal[e])
    fr10 = (2.0 ** (np.arange(NUM_FREQS, dtype=np.float32) - 1.0)).astype(
        np.float32)
    return dict(w0=w0, wh=wh, wo=wo, b0r=b0r, bhr=bhr, bor=bor,
                scal3=sc3, fr10=fr10)


def kernel(x, in_dim, layer_id, W0, b0, Wh, bh, scal, Wout, bout):
    from concourse.bass_utils import run_bass_kernel_spmd

    x = np.asarray(x, np.float32)
    in_dim = np.asarray(in_dim, np.float32)
    layer_id = np.asarray(layer_id)
    W0 = np.asarray(W0, np.float32)
    b0 = np.asarray(b0, np.float32)
    Wh = np.asarray(Wh, np.float32)
    bh = np.asarray(bh, np.float32)
    scal = np.asarray(scal, np.float32)
    Wout = np.asarray(Wout, np.float32)
    bout = np.asarray(bout, np.float32)

    # ---- dispatch: expert e -> cores 2e, 2e+1; pad to CAP per core ----
    PADIDX = N
    x_aug = np.vstack([x, np.ones((1, 4), np.float32)])
    d_aug = np.concatenate([in_dim, np.ones(1, np.float32)])
    perms = np.full((NCORE, CAP), PADIDX, np.int64)
    overflow = []
    for e in range(E):
        idx = np.flatnonzero(layer_id == e)
        if len(idx) > 2 * CAP:
            overflow.append(idx[2 * CAP:])
            idx = idx[:2 * CAP]
        nh = min((len(idx) + 1) // 2, CAP)
        perms[2 * e, :nh] = idx[:nh]
        perms[2 * e + 1, :len(idx) - nh] = idx[nh:]

    in_maps = []
    for c in range(NCORE):
        m = _prep_weights(c // 2, W0, b0, Wh, bh, scal, Wout, bout)
        p = perms[c]
        m["x_rows"] = np.ascontiguousarray(x_aug[p])
        m["indim_rows"] = np.ascontiguousarray(d_aug[p])
        in_maps.append(m)

    nc = _get_program()
    res = run_bass_kernel_spmd(nc, in_maps, core_ids=list(range(NCORE)),
                               **RUN_KWARGS)
    LAST_RESULT.clear()
    LAST_RESULT.append(res)

    out = np.zeros((N + 1, DOUT), np.float32)
    for c in range(NCORE):
        out[perms[c]] = np.asarray(res.results[c]["out_rows"], np.float32)

    # pathological overflow fallback (never hit for the benchmark input)
    if overflow:
        ov = np.concatenate(overflow)
        out[ov] = _numpy_ref(x[ov], in_dim[ov], layer_id[ov], W0, b0, Wh, bh,
                             scal, Wout, bout)
    return out[:N]


def _numpy_ref(x, in_dim, layer_id, W0, b0, Wh, bh, scal, Wout, bout):
    x = np.concatenate([x[:, :3] / x[:, 3:4], x[:, 3:]], axis=1)
    freqs = (2.0 ** np.arange(NUM_FREQS, dtype=np.float32)) * np.float32(np.pi)
    ang = x[:, None, :] * freqs[None, :, None]
    sc = np.stack([np.sin(ang), np.cos(ang)], axis=-1)
    xe = np.concatenate([x, sc.reshape(x.shape[0], -1)], axis=1)
    out = np.zeros((x.shape[0], DOUT), np.float32)
    for e in range(E):
        m = layer_id == e
        if not m.any():
            continue
        h = np.maximum(xe[m] @ W0[e] + b0[e], 0.0)
        for k in range(NL - 1):
            h = scal[e, k] * np.maximum(h @ Wh[e, k] + bh[e, k], 0.0) + h
        out[m] = h @ Wout[e] + bout[e]
    return out / in_dim[:, None]


# revision 35
# speedup vs baseline: 1.2357x; 1.1436x over previous
"""Trainium2 Bass kernel for nn_NeRF_MLP_Compose (MoE-routed NeRF MLP).

Strategy (v2):
  - Host-side MoE dispatch: each expert's rows are split across a PAIR of
    cores (core c handles expert c//2), so each core runs ONE expert dense
    over ~8.2k rows (CAP=8704 padded) and holds only that expert's weights.
  - bf16 weights + activations for all matmuls (tolerance is 2e-2); the
    positional-encoding angle path stays fp32 for phase accuracy.
  - Row-major front-end: normalize + angle/[mod 1]/sin/cos are computed with
    rows on partitions (no PE transposes, no PSUM copies); the encoded
    features are flipped to feature-major with the DMA XBAR transpose
    (16-bit, 16x128 tiles).  Output is flipped back the same way.
  - MLP: feature-major, K<=128 stationary blocks, N=512 moving tiles.
    Third residual folded into the out layer input (h3 = s2*t3 + h2).
  - Element-wise work is spread across ACT / DVE / GPSIMD so each engine's
    per-tile time roughly matches the PE's; the Tile list-scheduler
    overlaps tiles (all pools are multi-buffered).
"""
import sys
for _p in ("/opt/trn_rl_repo", "/root/.axon_site/_ro/trn_rl_repo"):
    if _p not in sys.path:
        sys.path.insert(0, _p)

import numpy as np
import ml_dtypes

N = 65536
E = 4            # experts
NCORE = 8
CAP = 8704       # rows per core (one expert per core pair; 2*CAP=17408 >> E[16384])
NT = 17          # 512-row tiles per core
R = 512          # rows per tile
C = 4            # 128-row chunks per tile
NUM_FREQS = 10
HID = 256
DOUT = 64
NL = 4           # layers -> 3 residual blocks
TWO_PI_F32 = float(np.float32(2 * np.pi))
HALF_PI_F32 = float(np.float32(0.5 * np.pi))
MAGIC_C = float(np.float32(1.5 * 2 ** 23))

_compiled = {}
RUN_KWARGS = {}    # test.py may set e.g. {"trace": True}
LAST_RESULT = []   # test.py reads the BassKernelResults appended here


def _build_program():
    import concourse.bass as bass
    from concourse import bacc
    import concourse.mybir as mybir
    import concourse.tile as tile

    F32 = mybir.dt.float32
    BF16 = mybir.dt.bfloat16
    P = 128
    ALU = mybir.AluOpType
    ACTF = mybir.ActivationFunctionType

    nc = bacc.Bacc("TRN2", target_bir_lowering=False, debug=False)

    # ---- DRAM I/O (per core; one expert's weights) ----
    x_d = nc.dram_tensor("x_rows", [CAP, 4], F32, kind="ExternalInput").ap()
    d_d = nc.dram_tensor("indim_rows", [CAP], F32, kind="ExternalInput").ap()
    fr_d = nc.dram_tensor("fr10", [NUM_FREQS], F32, kind="ExternalInput").ap()
    w0_d = nc.dram_tensor("w0", [85, HID], BF16, kind="ExternalInput").ap()
    wh_d = nc.dram_tensor("wh", [P, NL - 1, 2, 2, P], BF16,
                          kind="ExternalInput").ap()
    wo_d = nc.dram_tensor("wo", [P, 2, DOUT], BF16, kind="ExternalInput").ap()
    b0_d = nc.dram_tensor("b0r", [P, 2], F32, kind="ExternalInput").ap()
    bh_d = nc.dram_tensor("bhr", [P, NL - 1, 2], F32, kind="ExternalInput").ap()
    bo_d = nc.dram_tensor("bor", [DOUT, 1], F32, kind="ExternalInput").ap()
    sc_d = nc.dram_tensor("scal3", [NL - 1], F32, kind="ExternalInput").ap()
    out_d = nc.dram_tensor("out_rows", [CAP, DOUT], F32,
                           kind="ExternalOutput").ap()

    with tile.TileContext(nc) as tc:
        with tc.tile_pool(name="const", bufs=1) as cpool, \
             tc.tile_pool(name="fr", bufs=6) as fpool, \
             tc.tile_pool(name="act", bufs=12) as apool, \
             tc.tile_pool(name="psz", bufs=6, space="PSUM") as zpool, \
             tc.tile_pool(name="pso", bufs=2, space="PSUM") as opool:

            # ---- front-end constants first (the first tiles' front
            # chain must not queue behind the big weight DMAs) ----
            scl = cpool.tile([P, NL - 1], F32)
            nc.scalar.dma_start(
                out=scl,
                in_=bass.AP(tensor=sc_d.tensor, offset=0,
                            ap=[[0, P], [1, NL - 1]]))
            fr = cpool.tile([P, NUM_FREQS], F32)
            nc.scalar.dma_start(
                out=fr,
                in_=bass.AP(tensor=fr_d.tensor, offset=0,
                            ap=[[0, P], [1, NUM_FREQS]]))
            ph = cpool.tile([P, 2], F32)
            nc.vector.memset(ph[:, 0:1], 0.0)
            nc.vector.memset(ph[:, 1:2], 0.25)
            w0 = cpool.tile([85, HID], BF16)
            wh = cpool.tile([P, NL - 1, 2, 2, P], BF16)
            wo = cpool.tile([P, 2, DOUT], BF16)
            wos = cpool.tile([P, 2, DOUT], BF16)
            b0 = cpool.tile([P, 2], F32)
            bh = cpool.tile([P, NL - 1, 2], F32)
            bo = cpool.tile([DOUT, 1], F32)

            def load_weights():
                nc.scalar.dma_start(out=w0, in_=w0_d)
                nc.scalar.dma_start(out=wh, in_=wh_d)
                nc.scalar.dma_start(out=wo, in_=wo_d)
                nc.scalar.dma_start(out=b0, in_=b0_d)
                nc.scalar.dma_start(out=bh, in_=bh_d)
                nc.scalar.dma_start(out=bo, in_=bo_d)
                # s2-prescaled out weights: out = Wo^T h2 + (s2 Wo)^T t3 --
                # removes the third residual STT from the per-tile loop
                nc.vector.tensor_scalar_mul(wos, wo, scl[:, 2:3])

            def fronts(group):
                """Front-end for a group: one batched x/d load, then
                per-tile encode chains (small ops keep latency low)."""
                G = len(group)
                CG = G * C
                r0 = group[0] * R
                x_g = fpool.tile([P, CG, 4], F32, tag="x_t", bufs=2)
                nc.sync.dma_start(
                    out=x_g,
                    in_=bass.AP(tensor=x_d.tensor, offset=r0 * 4,
                                ap=[[4, P], [4 * P, CG], [1, 4]]))
                d_g = fpool.tile([P, CG], F32, tag="d_t", bufs=2)
                nc.sync.dma_start(
                    out=d_g,
                    in_=bass.AP(tensor=d_d.tensor, offset=r0,
                                ap=[[1, P], [P, CG]]))
                for gi, t in enumerate(group):
                    xe = front(x_g[:, gi * C:(gi + 1) * C, :])
                    st[t] = {"xe": xe, "dg": d_g, "gi": gi}

            def front(x_t):
                """Row-major front-end for one tile: normalize + encode."""
                rc = fpool.tile([P, C], F32, tag="rc")
                nc.vector.reciprocal(rc, x_t[:, :, 3])
                xn = fpool.tile([P, C, 4], F32, tag="xn")
                nc.gpsimd.tensor_mul(xn, x_t,
                                     rc[:, :, None].to_broadcast((P, C, 4)))
                nc.gpsimd.tensor_copy(xn[:, :, 3], x_t[:, :, 3])

                # angles in turns: t20[p, c, j, i] = x'_j * 2^(i-1) (exact);
                # t40 doubles it with the cos quarter-turn phase (folded in
                # BEFORE range reduction -- the Sin table domain is ~[-pi,pi])
                t20 = fpool.tile([P, C, 4, NUM_FREQS], F32, tag="t20")
                nc.gpsimd.tensor_mul(
                    t20, xn[:, :, :, None].to_broadcast((P, C, 4, NUM_FREQS)),
                    fr[:, None, None, :].to_broadcast((P, C, 4, NUM_FREQS)))
                t20f = t20.rearrange("p c j i -> p c (j i)")
                t40 = fpool.tile([P, C, 2, 40], F32, tag="t40")
                nc.gpsimd.tensor_tensor(
                    t40,
                    t20f[:, :, None, :].to_broadcast((P, C, 2, 40)),
                    ph[:, None, :, None].to_broadcast((P, C, 2, 40)),
                    ALU.add)
                # k = round(t40) via fp32 magic add; m40 = t40 - k in [-.5,.5]
                kt = fpool.tile([P, C, 2, 40], F32, tag="kt")
                nc.vector.tensor_scalar(kt, t40, MAGIC_C, MAGIC_C,
                                        ALU.add, ALU.subtract)
                m40 = fpool.tile([P, C, 2, 40], F32, tag="m40")
                nc.gpsimd.tensor_tensor(m40, t40, kt, ALU.subtract)

                # xe rows: [0:4]=x', [4:44]=sin, [44:84]=cos, [84]=1 (bias
                # row for the l0 matmul), [85:128]=junk
                xe_r = fpool.tile([P, C, P], BF16, tag="xe_r")
                nc.gpsimd.tensor_copy(xe_r[:, :, 0:4], xn)
                nc.gpsimd.memset(xe_r[:, :, 84:85], 1.0)
                m40f = m40.rearrange("p c s f -> p c (s f)")
                nc.scalar.activation(xe_r[:, :, 4:84], m40f, ACTF.Sin,
                                     bias=0.0, scale=TWO_PI_F32)

                # flip to feature-major via DMA XBAR transpose
                xe = apool.tile([P, R], BF16, tag="xe")
                nc.sync.dma_start(out=xe.rearrange("p (c q) -> p c q", c=C),
                                  in_=xe_r, transpose=True)
                return xe

            def l0_mm(xe):
                # bias rides the ones row -> biasless relus; z halves are
                # separate single-bank tiles so six tiles' worth of PSUM
                # can be in flight (3-way interleave)
                za = zpool.tile([P, R], F32, tag="z")
                nc.tensor.matmul(za, w0[:, 0:P], xe[0:85, :],
                                 start=True, stop=True)
                zb = zpool.tile([P, R], F32, tag="z")
                nc.tensor.matmul(zb, w0[:, P:HID], xe[0:85, :],
                                 start=True, stop=True)
                return za, zb

            def l0_relu(z0):
                za, zb = z0
                h = apool.tile([P, 2, R], BF16, tag="h")
                nc.scalar.activation(h[:, 0, :], za, ACTF.Relu,
                                     bias=0.0, scale=1.0)
                nc.vector.tensor_scalar(h[:, 1, :], zb, 0.0, None, ALU.max)
                return h

            def layer_mm(k, h):
                za = zpool.tile([P, R], F32, tag="z")
                nc.tensor.matmul(za, wh[:, k, 0, 0, :], h[:, 0, :],
                                 start=True, stop=False)
                nc.tensor.matmul(za, wh[:, k, 1, 0, :], h[:, 1, :],
                                 start=False, stop=True)
                zb = zpool.tile([P, R], F32, tag="z")
                nc.tensor.matmul(zb, wh[:, k, 0, 1, :], h[:, 0, :],
                                 start=True, stop=False)
                nc.tensor.matmul(zb, wh[:, k, 1, 1, :], h[:, 1, :],
                                 start=False, stop=True)
                return za, zb

            def layer_post(k, zk, h):
                # t = relu(zk + bh);  k<2: h' = s_k t + h;  k==2: keep t3
                # (its residual is folded into the prescaled out weights)
                za, zb = zk
                tt = apool.tile([P, 2, R], BF16, tag="t")
                nc.scalar.activation(tt[:, 0, :], za, ACTF.Relu,
                                     bias=bh[:, k, 0:1], scale=1.0)
                if k < 2:
                    nc.scalar.activation(tt[:, 1, :], zb, ACTF.Relu,
                                         bias=bh[:, k, 1:2], scale=1.0)
                else:
                    nc.vector.tensor_scalar(tt[:, 1, :], zb,
                                            bh[:, k, 1:2], 0.0,
                                            ALU.add, ALU.max)
                if k == 2:
                    return h, tt
                h_new = apool.tile([P, 2, R], BF16, tag="h")
                nc.vector.scalar_tensor_tensor(h_new, tt, scl[:, k:k + 1],
                                               h, ALU.mult, ALU.add)
                return h_new, None

            def out_mm_h2(h2):
                o_ps = opool.tile([DOUT, R], F32, tag="o")
                nc.tensor.matmul(o_ps, wo[:, 0, :], h2[:, 0, :],
                                 start=True, stop=False)
                nc.tensor.matmul(o_ps, wo[:, 1, :], h2[:, 1, :],
                                 start=False, stop=False)
                return o_ps

            def out_mm_t3(o_ps, t3):
                nc.tensor.matmul(o_ps, wos[:, 0, :], t3[:, 0, :],
                                 start=False, stop=False)
                nc.tensor.matmul(o_ps, wos[:, 1, :], t3[:, 1, :],
                                 start=False, stop=True)

            def epilogue(t, o_ps, d2):
                r0 = t * R
                oT = fpool.tile([DOUT, R], BF16, tag="oT")
                nc.scalar.activation(oT, o_ps, ACTF.Identity,
                                     bias=bo, scale=1.0)
                # flip back to row-major, divide by in_dim, store
                o_r = fpool.tile([P, C, DOUT], BF16, tag="o_r")
                nc.scalar.dma_start(out=o_r, in_=oT, transpose=True)
                rid = fpool.tile([P, C], F32, tag="rid")
                nc.vector.reciprocal(rid, d2)
                o_f = fpool.tile([P, C, DOUT], F32, tag="o_f")
                nc.gpsimd.tensor_mul(
                    o_f, o_r, rid[:, :, None].to_broadcast((P, C, DOUT)))
                nc.scalar.dma_start(
                    out=bass.AP(tensor=out_d.tensor, offset=r0 * DOUT,
                                ap=[[DOUT, P], [P * DOUT, C], [1, DOUT]]),
                    in_=o_f)

            # two/three tiles interleaved per layer so the PE always has a
            # ready matmul burst while another tile's relu/residual chain
            # runs; the next group's front-end is emitted before this
            # group's hidden layers to fill engine idle
            groups = [[3 * g, 3 * g + 1, 3 * g + 2] for g in range(5)]
            groups.append([15, 16])

            st = {}
            fronts(groups[0])
            load_weights()
            for pi, group in enumerate(groups):
                for t in group:
                    st[t]["z"] = l0_mm(st[t]["xe"])
                for t in group:
                    st[t]["h"] = l0_relu(st[t]["z"])
                if pi + 1 < len(groups):
                    fronts(groups[pi + 1])
                for k in range(NL - 1):
                    for t in group:
                        st[t]["zk"] = layer_mm(k, st[t]["h"])
                    if k == 2:
                        # out-layer h2 part fills the PE gap while the k2
                        # relu chain runs on ACT/DVE
                        for t in group:
                            st[t]["o"] = out_mm_h2(st[t]["h"])
                    for t in group:
                        st[t]["h"], st[t]["t3"] = layer_post(
                            k, st[t]["zk"], st[t]["h"])
                for t in group:
                    out_mm_t3(st[t]["o"], st[t]["t3"])
                for t in group:
                    gi = st[t]["gi"]
                    d2 = st[t]["dg"][:, gi * C:(gi + 1) * C]
                    epilogue(t, st[t]["o"], d2)

    nc.compile()
    return nc


def _get_program():
    if "nc" not in _compiled:
        _compiled["nc"] = _build_program()
    return _compiled["nc"]


def _xe_perm():
    """perm[slot] = reference xe column for device slot order
    (slots: 0..3 = x', 4 + j*10 + i = sin, 44 + j*10 + i = cos)."""
    perm = np.zeros(84, np.int64)
    perm[0:4] = np.arange(4)
    for s in range(2):
        for j in range(4):
            for i in range(NUM_FREQS):
                perm[4 + s * 40 + j * 10 + i] = 4 + i * 8 + j * 2 + s
    return perm


def _prep_weights(e, W0, b0, Wh, bh, scal, Wout, bout):
    """Host-side layout transforms (permutation / reshape / cast only)."""
    bf = ml_dtypes.bfloat16
    w0 = np.ascontiguousarray(
        np.vstack([W0[e][_xe_perm()], b0[e][None, :]])).astype(bf)  # [85,256]
    wh = np.ascontiguousarray(
        Wh[e].reshape(NL - 1, 2, 128, 2, 128)
        .transpose(2, 0, 1, 3, 4)).astype(bf)                      # [128,3,2,2,128]
    wo = np.ascontiguousarray(
        Wout[e].reshape(2, 128, DOUT).transpose(1, 0, 2)).astype(bf)
    b0r = np.ascontiguousarray(b0[e].reshape(2, 128).T)            # [128,2]
    bhr = np.ascontiguousarray(
        bh[e].reshape(NL - 1, 2, 128).transpose(2, 0, 1))          # [128,3,2]
    bor = np.ascontiguousarray(bout[e].reshape(DOUT, 1))
    sc3 = np.ascontiguousarray(scal[e])
    fr10 = (2.0 ** (np.arange(NUM_FREQS, dtype=np.float32) - 1.0)).astype(
        np.float32)
    return dict(w0=w0, wh=wh, wo=wo, b0r=b0r, bhr=bhr, bor=bor,
                scal3=sc3, fr10=fr10)


def kernel(x, in_dim, layer_id, W0, b0, Wh, bh, scal, Wout, bout):
    from concourse.bass_utils import run_bass_kernel_spmd

    x = np.asarray(x, np.float32)
    in_dim = np.asarray(in_dim, np.float32)
    layer_id = np.asarray(layer_id)
    W0 = np.asarray(W0, np.float32)
    b0 = np.asarray(b0, np.float32)
    Wh = np.asarray(Wh, np.float32)
    bh = np.asarray(bh, np.float32)
    scal = np.asarray(scal, np.float32)
    Wout = np.asarray(Wout, np.float32)
    bout = np.asarray(bout, np.float32)

    # ---- dispatch: expert e -> cores 2e, 2e+1; pad to CAP per core ----
    PADIDX = N
    x_aug = np.vstack([x, np.ones((1, 4), np.float32)])
    d_aug = np.concatenate([in_dim, np.ones(1, np.float32)])
    perms = np.full((NCORE, CAP), PADIDX, np.int64)
    overflow = []
    for e in range(E):
        idx = np.flatnonzero(layer_id == e)
        if len(idx) > 2 * CAP:
            overflow.append(idx[2 * CAP:])
            idx = idx[:2 * CAP]
        nh = min((len(idx) + 1) // 2, CAP)
        perms[2 * e, :nh] = idx[:nh]
        perms[2 * e + 1, :len(idx) - nh] = idx[nh:]

    in_maps = []
    for c in range(NCORE):
        m = _prep_weights(c // 2, W0, b0, Wh, bh, scal, Wout, bout)
        p = perms[c]
        m["x_rows"] = np.ascontiguousarray(x_aug[p])
        m["indim_rows"] = np.ascontiguousarray(d_aug[p])
        in_maps.append(m)

    nc = _get_program()
    res = run_bass_kernel_spmd(nc, in_maps, core_ids=list(range(NCORE)),
                               **RUN_KWARGS)
    LAST_RESULT.clear()
    LAST_RESULT.append(res)

    out = np.zeros((N + 1, DOUT), np.float32)
    for c in range(NCORE):
        out[perms[c]] = np.asarray(res.results[c]["out_rows"], np.float32)

    # pathological overflow fallback (never hit for the benchmark input)
    if overflow:
        ov = np.concatenate(overflow)
        out[ov] = _numpy_ref(x[ov], in_dim[ov], layer_id[ov], W0, b0, Wh, bh,
                             scal, Wout, bout)
    return out[:N]


def _numpy_ref(x, in_dim, layer_id, W0, b0, Wh, bh, scal, Wout, bout):
    x = np.concatenate([x[:, :3] / x[:, 3:4], x[:, 3:]], axis=1)
    freqs = (2.0 ** np.arange(NUM_FREQS, dtype=np.float32)) * np.float32(np.pi)
    ang = x[:, None, :] * freqs[None, :, None]
    sc = np.stack([np.sin(ang), np.cos(ang)], axis=-1)
    xe = np.concatenate([x, sc.reshape(x.shape[0], -1)], axis=1)
    out = np.zeros((x.shape[0], DOUT), np.float32)
    for e in range(E):
        m = layer_id == e
        if not m.any():
            continue
        h = np.maximum(xe[m] @ W0[e] + b0[e], 0.0)
        for k in range(NL - 1):
            h = scal[e, k] * np.maximum(h @ Wh[e, k] + bh[e, k], 0.0) + h
        out[m] = h @ Wout[e] + bout[e]
    return out / in_dim[:, None]


# revision 36
# speedup vs baseline: 1.4645x; 1.1852x over previous
"""Trainium2 Bass kernel for nn_NeRF_MLP_Compose (MoE-routed NeRF MLP).

Strategy (v2):
  - Host-side MoE dispatch: each expert's rows are split across a PAIR of
    cores (core c handles expert c//2), so each core runs ONE expert dense
    over ~8.2k rows (CAP=8704 padded) and holds only that expert's weights.
  - bf16 weights + activations for all matmuls (tolerance is 2e-2); the
    positional-encoding angle path stays fp32 for phase accuracy.
  - Row-major front-end: normalize + angle/[mod 1]/sin/cos are computed with
    rows on partitions (no PE transposes, no PSUM copies); the encoded
    features are flipped to feature-major with the DMA XBAR transpose
    (16-bit, 16x128 tiles).  Output is flipped back the same way.
  - MLP: feature-major, K<=128 stationary blocks, N=512 moving tiles.
    Third residual folded into the out layer input (h3 = s2*t3 + h2).
  - Element-wise work is spread across ACT / DVE / GPSIMD so each engine's
    per-tile time roughly matches the PE's; the Tile list-scheduler
    overlaps tiles (all pools are multi-buffered).
"""
import sys
for _p in ("/opt/trn_rl_repo", "/root/.axon_site/_ro/trn_rl_repo"):
    if _p not in sys.path:
        sys.path.insert(0, _p)

import numpy as np
import ml_dtypes

N = 65536
E = 4            # experts
NCORE = 8
CAP = 8704       # rows per core (one expert per core pair; 2*CAP=17408 >> E[16384])
NT = 17          # 512-row tiles per core
R = 512          # rows per tile
C = 4            # 128-row chunks per tile
NUM_FREQS = 10
HID = 256
DOUT = 64
NL = 4           # layers -> 3 residual blocks
TWO_PI_F32 = float(np.float32(2 * np.pi))
HALF_PI_F32 = float(np.float32(0.5 * np.pi))
MAGIC_C = float(np.float32(1.5 * 2 ** 23))

_compiled = {}
RUN_KWARGS = {}    # test.py may set e.g. {"trace": True}
LAST_RESULT = []   # test.py reads the BassKernelResults appended here


def _build_program():
    import concourse.bass as bass
    from concourse import bacc
    import concourse.mybir as mybir
    import concourse.tile as tile

    F32 = mybir.dt.float32
    BF16 = mybir.dt.bfloat16
    P = 128
    ALU = mybir.AluOpType
    ACTF = mybir.ActivationFunctionType

    nc = bacc.Bacc("TRN2", target_bir_lowering=False, debug=False)

    # ---- DRAM I/O (per core; one expert's weights) ----
    x_d = nc.dram_tensor("x_rows", [CAP, 4], F32, kind="ExternalInput").ap()
    d_d = nc.dram_tensor("indim_rows", [CAP], F32, kind="ExternalInput").ap()
    fr_d = nc.dram_tensor("fr10", [NUM_FREQS], F32, kind="ExternalInput").ap()
    w0_d = nc.dram_tensor("w0", [85, HID], BF16, kind="ExternalInput").ap()
    wh_d = nc.dram_tensor("wh", [P, NL - 1, 2, 2, P], BF16,
                          kind="ExternalInput").ap()
    wo_d = nc.dram_tensor("wo", [P, 2, DOUT], BF16, kind="ExternalInput").ap()
    b0_d = nc.dram_tensor("b0r", [P, 2], F32, kind="ExternalInput").ap()
    bh_d = nc.dram_tensor("bhr", [P, NL - 1, 2], F32, kind="ExternalInput").ap()
    bo_d = nc.dram_tensor("bor", [DOUT, 1], F32, kind="ExternalInput").ap()
    sc_d = nc.dram_tensor("scal3", [NL - 1], F32, kind="ExternalInput").ap()
    out_d = nc.dram_tensor("out_rows", [CAP, DOUT], F32,
                           kind="ExternalOutput").ap()

    with tile.TileContext(nc) as tc:
        with tc.tile_pool(name="const", bufs=1) as cpool, \
             tc.tile_pool(name="fr", bufs=6) as fpool, \
             tc.tile_pool(name="act", bufs=12) as apool, \
             tc.tile_pool(name="psz", bufs=6, space="PSUM") as zpool, \
             tc.tile_pool(name="pso", bufs=2, space="PSUM") as opool:

            # ---- front-end constants first (the first tiles' front
            # chain must not queue behind the big weight DMAs) ----
            scl = cpool.tile([P, NL - 1], F32)
            nc.scalar.dma_start(
                out=scl,
                in_=bass.AP(tensor=sc_d.tensor, offset=0,
                            ap=[[0, P], [1, NL - 1]]))
            fr = cpool.tile([P, NUM_FREQS], F32)
            nc.scalar.dma_start(
                out=fr,
                in_=bass.AP(tensor=fr_d.tensor, offset=0,
                            ap=[[0, P], [1, NUM_FREQS]]))
            ph = cpool.tile([P, 2], F32)
            nc.vector.memset(ph[:, 0:1], 0.0)
            nc.vector.memset(ph[:, 1:2], 0.25)
            w0 = cpool.tile([85, HID], BF16)
            wh = cpool.tile([P, NL - 1, 2, 2, P], BF16)
            wo = cpool.tile([P, 2, DOUT], BF16)
            wos = cpool.tile([P, 2, DOUT], BF16)
            b0 = cpool.tile([P, 2], F32)
            bh = cpool.tile([P, NL - 1, 2], F32)
            bo = cpool.tile([DOUT, 1], F32)

            def load_weights():
                nc.scalar.dma_start(out=w0, in_=w0_d)
                nc.scalar.dma_start(out=wh, in_=wh_d)
                nc.scalar.dma_start(out=wo, in_=wo_d)
                nc.scalar.dma_start(out=b0, in_=b0_d)
                nc.scalar.dma_start(out=bh, in_=bh_d)
                nc.scalar.dma_start(out=bo, in_=bo_d)
                # s2-prescaled out weights: out = Wo^T h2 + (s2 Wo)^T t3 --
                # removes the third residual STT from the per-tile loop
                nc.vector.tensor_scalar_mul(wos, wo, scl[:, 2:3])

            def fronts(group):
                """Front-end for a group: one batched x/d load, then
                per-tile encode chains (small ops keep latency low)."""
                G = len(group)
                CG = G * C
                r0 = group[0] * R
                x_g = fpool.tile([P, CG, 4], F32, tag="x_t", bufs=2)
                nc.sync.dma_start(
                    out=x_g,
                    in_=bass.AP(tensor=x_d.tensor, offset=r0 * 4,
                                ap=[[4, P], [4 * P, CG], [1, 4]]))
                d_g = fpool.tile([P, CG], F32, tag="d_t", bufs=2)
                nc.sync.dma_start(
                    out=d_g,
                    in_=bass.AP(tensor=d_d.tensor, offset=r0,
                                ap=[[1, P], [P, CG]]))
                for gi, t in enumerate(group):
                    xe = front(x_g[:, gi * C:(gi + 1) * C, :], t < 6)
                    st[t] = {"xe": xe, "dg": d_g, "gi": gi}

            def front(x_t, ramp=False):
                """Row-major front-end for one tile: normalize + encode.
                During the pipeline ramp the heavy stages run on the (then
                idle) DVE instead of the slower GPSIMD."""
                ve = nc.vector if ramp else nc.gpsimd
                rc = fpool.tile([P, C], F32, tag="rc")
                nc.vector.reciprocal(rc, x_t[:, :, 3])
                xn = fpool.tile([P, C, 4], F32, tag="xn")
                nc.gpsimd.tensor_mul(xn, x_t,
                                     rc[:, :, None].to_broadcast((P, C, 4)))
                nc.gpsimd.tensor_copy(xn[:, :, 3], x_t[:, :, 3])

                # angles in turns: t20[p, c, j, i] = x'_j * 2^(i-1) (exact);
                # t40 doubles it with the cos quarter-turn phase (folded in
                # BEFORE range reduction -- the Sin table domain is ~[-pi,pi])
                t20 = fpool.tile([P, C, 4, NUM_FREQS], F32, tag="t20")
                ve.tensor_mul(
                    t20, xn[:, :, :, None].to_broadcast((P, C, 4, NUM_FREQS)),
                    fr[:, None, None, :].to_broadcast((P, C, 4, NUM_FREQS)))
                t20f = t20.rearrange("p c j i -> p c (j i)")
                t40 = fpool.tile([P, C, 2, 40], F32, tag="t40")
                ve.tensor_tensor(
                    t40,
                    t20f[:, :, None, :].to_broadcast((P, C, 2, 40)),
                    ph[:, None, :, None].to_broadcast((P, C, 2, 40)),
                    ALU.add)
                # k = round(t40) via fp32 magic add; m40 = t40 - k in [-.5,.5]
                kt = fpool.tile([P, C, 2, 40], F32, tag="kt")
                nc.vector.tensor_scalar(kt, t40, MAGIC_C, MAGIC_C,
                                        ALU.add, ALU.subtract)
                m40 = fpool.tile([P, C, 2, 40], F32, tag="m40")
                ve.tensor_tensor(m40, t40, kt, ALU.subtract)

                # xe rows: [0:4]=x', [4:44]=sin, [44:84]=cos, [84]=1 (bias
                # row for the l0 matmul), [85:128]=junk
                xe_r = fpool.tile([P, C, P], BF16, tag="xe_r")
                nc.gpsimd.tensor_copy(xe_r[:, :, 0:4], xn)
                nc.gpsimd.memset(xe_r[:, :, 84:85], 1.0)
                m40f = m40.rearrange("p c s f -> p c (s f)")
                nc.scalar.activation(xe_r[:, :, 4:84], m40f, ACTF.Sin,
                                     bias=0.0, scale=TWO_PI_F32)

                # flip to feature-major via DMA XBAR transpose
                xe = apool.tile([P, R], BF16, tag="xe")
                nc.sync.dma_start(out=xe.rearrange("p (c q) -> p c q", c=C),
                                  in_=xe_r, transpose=True)
                return xe

            def l0_mm(xe):
                # bias rides the ones row -> biasless relus; z halves are
                # separate single-bank tiles so six tiles' worth of PSUM
                # can be in flight (3-way interleave)
                za = zpool.tile([P, R], F32, tag="z")
                nc.tensor.matmul(za, w0[:, 0:P], xe[0:85, :],
                                 start=True, stop=True)
                zb = zpool.tile([P, R], F32, tag="z")
                nc.tensor.matmul(zb, w0[:, P:HID], xe[0:85, :],
                                 start=True, stop=True)
                return za, zb

            def l0_relu(z0):
                za, zb = z0
                h = apool.tile([P, 2, R], BF16, tag="h")
                nc.scalar.activation(h[:, 0, :], za, ACTF.Relu,
                                     bias=0.0, scale=1.0)
                nc.vector.tensor_scalar(h[:, 1, :], zb, 0.0, None, ALU.max)
                return h

            def layer_mm(k, h):
                za = zpool.tile([P, R], F32, tag="z")
                nc.tensor.matmul(za, wh[:, k, 0, 0, :], h[:, 0, :],
                                 start=True, stop=False)
                nc.tensor.matmul(za, wh[:, k, 1, 0, :], h[:, 1, :],
                                 start=False, stop=True)
                zb = zpool.tile([P, R], F32, tag="z")
                nc.tensor.matmul(zb, wh[:, k, 0, 1, :], h[:, 0, :],
                                 start=True, stop=False)
                nc.tensor.matmul(zb, wh[:, k, 1, 1, :], h[:, 1, :],
                                 start=False, stop=True)
                return za, zb

            def layer_post(k, zk, h, t):
                # t = relu(zk + bh);  k<2: h' = s_k t + h;  k==2: keep t3
                # (its residual is folded into the prescaled out weights)
                za, zb = zk
                tt = apool.tile([P, 2, R], BF16, tag="t")
                nc.scalar.activation(tt[:, 0, :], za, ACTF.Relu,
                                     bias=bh[:, k, 0:1], scale=1.0)
                if k == 1 or (k == 0 and t % 2 == 1):
                    nc.scalar.activation(tt[:, 1, :], zb, ACTF.Relu,
                                         bias=bh[:, k, 1:2], scale=1.0)
                else:
                    nc.vector.tensor_scalar(tt[:, 1, :], zb,
                                            bh[:, k, 1:2], 0.0,
                                            ALU.add, ALU.max)
                if k == 2:
                    return h, tt
                h_new = apool.tile([P, 2, R], BF16, tag="h")
                nc.vector.scalar_tensor_tensor(h_new, tt, scl[:, k:k + 1],
                                               h, ALU.mult, ALU.add)
                return h_new, None

            def out_mm_h2(h2):
                o_ps = opool.tile([DOUT, R], F32, tag="o")
                nc.tensor.matmul(o_ps, wo[:, 0, :], h2[:, 0, :],
                                 start=True, stop=False)
                nc.tensor.matmul(o_ps, wo[:, 1, :], h2[:, 1, :],
                                 start=False, stop=False)
                return o_ps

            def out_mm_t3(o_ps, t3):
                nc.tensor.matmul(o_ps, wos[:, 0, :], t3[:, 0, :],
                                 start=False, stop=False)
                nc.tensor.matmul(o_ps, wos[:, 1, :], t3[:, 1, :],
                                 start=False, stop=True)

            def epilogue(t, o_ps, d2):
                r0 = t * R
                oT = fpool.tile([DOUT, R], BF16, tag="oT")
                nc.scalar.activation(oT, o_ps, ACTF.Identity,
                                     bias=bo, scale=1.0)
                # flip back to row-major, divide by in_dim, store
                o_r = fpool.tile([P, C, DOUT], BF16, tag="o_r")
                nc.sync.dma_start(out=o_r, in_=oT, transpose=True)
                rid = fpool.tile([P, C], F32, tag="rid")
                nc.vector.reciprocal(rid, d2)
                o_f = fpool.tile([P, C, DOUT], F32, tag="o_f")
                nc.gpsimd.tensor_mul(
                    o_f, o_r, rid[:, :, None].to_broadcast((P, C, DOUT)))
                nc.sync.dma_start(
                    out=bass.AP(tensor=out_d.tensor, offset=r0 * DOUT,
                                ap=[[DOUT, P], [P * DOUT, C], [1, DOUT]]),
                    in_=o_f)

            # two/three tiles interleaved per layer so the PE always has a
            # ready matmul burst while another tile's relu/residual chain
            # runs; the next group's front-end is emitted before this
            # group's hidden layers to fill engine idle
            groups = [[3 * g, 3 * g + 1, 3 * g + 2] for g in range(5)]
            groups.append([15, 16])

            st = {}
            fronts(groups[0])
            load_weights()
            for pi, group in enumerate(groups):
                for t in group:
                    st[t]["z"] = l0_mm(st[t]["xe"])
                for t in group:
                    st[t]["h"] = l0_relu(st[t]["z"])
                if pi + 1 < len(groups):
                    fronts(groups[pi + 1])
                for k in range(NL - 1):
                    for t in group:
                        st[t]["zk"] = layer_mm(k, st[t]["h"])
                    if k == 2:
                        # out-layer h2 part fills the PE gap while the k2
                        # relu chain runs on ACT/DVE
                        for t in group:
                            st[t]["o"] = out_mm_h2(st[t]["h"])
                    for t in group:
                        st[t]["h"], st[t]["t3"] = layer_post(
                            k, st[t]["zk"], st[t]["h"], t)
                for t in group:
                    out_mm_t3(st[t]["o"], st[t]["t3"])
                for t in group:
                    gi = st[t]["gi"]
                    d2 = st[t]["dg"][:, gi * C:(gi + 1) * C]
                    epilogue(t, st[t]["o"], d2)

    nc.compile()
    return nc


def _get_program():
    if "nc" not in _compiled:
        _compiled["nc"] = _build_program()
    return _compiled["nc"]


def _xe_perm():
    """perm[slot] = reference xe column for device slot order
    (slots: 0..3 = x', 4 + j*10 + i = sin, 44 + j*10 + i = cos)."""
    perm = np.zeros(84, np.int64)
    perm[0:4] = np.arange(4)
    for s in range(2):
        for j in range(4):
            for i in range(NUM_FREQS):
                perm[4 + s * 40 + j * 10 + i] = 4 + i * 8 + j * 2 + s
    return perm


def _prep_weights(e, W0, b0, Wh, bh, scal, Wout, bout):
    """Host-side layout transforms (permutation / reshape / cast only)."""
    bf = ml_dtypes.bfloat16
    w0 = np.ascontiguousarray(
        np.vstack([W0[e][_xe_perm()], b0[e][None, :]])).astype(bf)  # [85,256]
    wh = np.ascontiguousarray(
        Wh[e].reshape(NL - 1, 2, 128, 2, 128)
        .transpose(2, 0, 1, 3, 4)).astype(bf)                      # [128,3,2,2,128]
    wo = np.ascontiguousarray(
        Wout[e].reshape(2, 128, DOUT).transpose(1, 0, 2)).astype(bf)
    b0r = np.ascontiguousarray(b0[e].reshape(2, 128).T)            # [128,2]
    bhr = np.ascontiguousarray(
        bh[e].reshape(NL - 1, 2, 128).transpose(2, 0, 1))          # [128,3,2]
    bor = np.ascontiguousarray(bout[e].reshape(DOUT, 1))
    sc3 = np.ascontiguousarray(scal[e])
    fr10 = (2.0 ** (np.arange(NUM_FREQS, dtype=np.float32) - 1.0)).astype(
        np.float32)
    return dict(w0=w0, wh=wh, wo=wo, b0r=b0r, bhr=bhr, bor=bor,
                scal3=sc3, fr10=fr10)


def kernel(x, in_dim, layer_id, W0, b0, Wh, bh, scal, Wout, bout):
    from concourse.bass_utils import run_bass_kernel_spmd

    x = np.asarray(x, np.float32)
    in_dim = np.asarray(in_dim, np.float32)
    layer_id = np.asarray(layer_id)
    W0 = np.asarray(W0, np.float32)
    b0 = np.asarray(b0, np.float32)
    Wh = np.asarray(Wh, np.float32)
    bh = np.asarray(bh, np.float32)
    scal = np.asarray(scal, np.float32)
    Wout = np.asarray(Wout, np.float32)
    bout = np.asarray(bout, np.float32)

    # ---- dispatch: expert e -> cores 2e, 2e+1; pad to CAP per core ----
    PADIDX = N
    x_aug = np.vstack([x, np.ones((1, 4), np.float32)])
    d_aug = np.concatenate([in_dim, np.ones(1, np.float32)])
    perms = np.full((NCORE, CAP), PADIDX, np.int64)
    overflow = []
    for e in range(E):
        idx = np.flatnonzero(layer_id == e)
        if len(idx) > 2 * CAP:
            overflow.append(idx[2 * CAP:])
            idx = idx[:2 * CAP]
        nh = min((len(idx) + 1) // 2, CAP)
        perms[2 * e, :nh] = idx[:nh]
        perms[2 * e + 1, :len(idx) - nh] = idx[nh:]

    in_maps = []
    for c in range(NCORE):
        m = _prep_weights(c // 2, W0, b0, Wh, bh, scal, Wout, bout)
        p = perms[c]
        m["x_rows"] = np.ascontiguousarray(x_aug[p])
        m["indim_rows"] = np.ascontiguousarray(d_aug[p])
        in_maps.append(m)

    nc = _get_program()
    res = run_bass_kernel_spmd(nc, in_maps, core_ids=list(range(NCORE)),
                               **RUN_KWARGS)
    LAST_RESULT.clear()
    LAST_RESULT.append(res)

    out = np.zeros((N + 1, DOUT), np.float32)
    for c in range(NCORE):
        out[perms[c]] = np.asarray(res.results[c]["out_rows"], np.float32)

    # pathological overflow fallback (never hit for the benchmark input)
    if overflow:
        ov = np.concatenate(overflow)
        out[ov] = _numpy_ref(x[ov], in_dim[ov], layer_id[ov], W0, b0, Wh, bh,
                             scal, Wout, bout)
    return out[:N]


def _numpy_ref(x, in_dim, layer_id, W0, b0, Wh, bh, scal, Wout, bout):
    x = np.concatenate([x[:, :3] / x[:, 3:4], x[:, 3:]], axis=1)
    freqs = (2.0 ** np.arange(NUM_FREQS, dtype=np.float32)) * np.float32(np.pi)
    ang = x[:, None, :] * freqs[None, :, None]
    sc = np.stack([np.sin(ang), np.cos(ang)], axis=-1)
    xe = np.concatenate([x, sc.reshape(x.shape[0], -1)], axis=1)
    out = np.zeros((x.shape[0], DOUT), np.float32)
    for e in range(E):
        m = layer_id == e
        if not m.any():
            continue
        h = np.maximum(xe[m] @ W0[e] + b0[e], 0.0)
        for k in range(NL - 1):
            h = scal[e, k] * np.maximum(h @ Wh[e, k] + bh[e, k], 0.0) + h
        out[m] = h @ Wout[e] + bout[e]
    return out / in_dim[:, None]
